# revision 7
# baseline (speedup 1.0000x reference)
"""Bahdanau attention Trainium2 kernel.

Reference computation (per batch row b):
    q_proj = query @ W1 + b1                    # [U]
    v_proj = values @ W2 + b2                   # [T, U]
    scores = tanh(q_proj + v_proj)              # [T, U]
    logits = scores @ V (+ bv, cancels in softmax)   # [T]
    aw     = softmax(logits over T)             # [T]
    context = sum_t aw[t] * values[t]           # [D]

Sharding: data-parallel over batch B=32 across 8 cores (4 rows each).

Per-core kernel layout ([u, t] "transposed" score layout so the q_proj+b
bias is a per-partition ACT bias and the V reduction is a PE matmul):
  - W2 resident in SBUF as [128, dc, u]; values streamed in T-chunks,
    transposed t->partition via PE transpose (identity matmul).
  - v_projT chunk [u=128, t=512] accumulated in PSUM over 8 d-chunks.
  - ACT tanh with per-partition bias (q_proj+b1+b2) -> scoresT in SBUF.
  - logits chunk [1, 512] = V-chunk.T @ scoresT accumulated over u-chunks.
  - softmax on [1, 2048] (ACT exp with accum_out for the sum).
  - weights transposed to [t=partition, 1] via K=1 matmul with ones.
  - context [1, D] = w.T @ values accumulated over 16 t-subtiles (2nd
    streaming pass over values).
Matmuls run as float32r (full PE rate at N>=256) unless USE_BF16.
"""

import sys

if "/opt/trn_rl_repo" not in sys.path:
    sys.path.insert(0, "/opt/trn_rl_repo")

import numpy as np

import concourse.bacc as bacc
import concourse.mybir as mybir
from concourse.bass_utils import run_bass_kernel_spmd
from concourse.masks import make_identity
from concourse.tile import TileContext

F32 = mybir.dt.float32
F32R = mybir.dt.float32r
BF16 = mybir.dt.bfloat16
TANH = mybir.ActivationFunctionType.Tanh
EXP = mybir.ActivationFunctionType.Exp
AXX = mybir.AxisListType.X

N_CORES = 8
USE_BF16 = False


def build_nc(nb, t_len, d, u, tc, use_bf16, debug_taps=False, n_devices=None):
    """Build the per-core Bass program. nb: batch rows per core."""
    assert t_len % tc == 0 and tc % 128 == 0 and d % 128 == 0 and u % 128 == 0
    assert tc <= 512 and d % 512 == 0
    ntc = t_len // tc      # T chunks
    s_sub = tc // 128      # 128-row subtiles per T chunk
    dc_n = d // 128        # d chunks (contraction)
    uc_n = u // 128        # u chunks
    nt128 = t_len // 128   # 128-col chunks of the weight vector

    mmdt = BF16 if use_bf16 else F32R   # main matmul operand dtype
    vdt = F32 if use_bf16 else F32R     # dram dtype for tensors feeding f32r

    nc = bacc.Bacc("TRN2", target_bir_lowering=False, debug=False,
                   num_devices=n_devices or N_CORES)

    query = nc.dram_tensor("query", [nb, d], F32, kind="ExternalInput").ap()
    values = nc.dram_tensor("values", [nb, t_len, d], vdt,
                            kind="ExternalInput").ap()
    w1 = nc.dram_tensor("W1", [d, u], F32, kind="ExternalInput").ap()
    b1 = nc.dram_tensor("b1", [u], F32, kind="ExternalInput").ap()
    w2 = nc.dram_tensor("W2", [d, u], vdt, kind="ExternalInput").ap()
    b2 = nc.dram_tensor("b2", [u], F32, kind="ExternalInput").ap()
    v_w = nc.dram_tensor("V", [u, 1], F32R, kind="ExternalInput").ap()
    ctx_out = nc.dram_tensor("ctx_out", [nb, d], F32,
                             kind="ExternalOutput").ap()
    aw_out = nc.dram_tensor("aw_out", [nb, t_len], F32,
                            kind="ExternalOutput").ap()
    if debug_taps:
        dbg_logits = nc.dram_tensor("dbg_logits", [nb, t_len], F32,
                                    kind="ExternalOutput").ap()
        dbg_sume = nc.dram_tensor("dbg_sume", [nb, 2], F32,
                                  kind="ExternalOutput").ap()
        dbg_bias = nc.dram_tensor("dbg_bias", [128, u // 128, nb], F32,
                                  kind="ExternalOutput").ap()
        dbg_vt = nc.dram_tensor("dbg_vt", [128, d // 128, tc], F32,
                                kind="ExternalOutput").ap()
        dbg_sc = nc.dram_tensor("dbg_sc", [128, tc], F32,
                                kind="ExternalOutput").ap()

    with TileContext(nc) as tc_:
        with tc_.tile_pool(name="consts", bufs=1) as consts:
            ident_f32 = consts.tile([128, 128], F32)
            make_identity(nc, ident_f32)
            if mmdt == F32:
                ident = ident_f32
            else:
                ident = consts.tile([128, 128], mmdt)
                nc.vector.tensor_copy(ident, ident_f32)
            idf4 = consts.tile([nb, nb], F32)
            make_identity(nc, idf4)
            ones11 = consts.tile([1, 1], F32)
            nc.vector.memset(ones11, 1.0)

            w2sb = consts.tile([128, dc_n, u], mmdt)
            w2_src = w2.rearrange("(c p) u -> p c u", p=128)
            if use_bf16:
                nc.gpsimd.dma_start(out=w2sb, in_=w2_src)
            else:
                nc.sync.dma_start(out=w2sb, in_=w2_src)

            vsb = consts.tile([128, uc_n], F32R)
            nc.sync.dma_start(out=vsb, in_=v_w.rearrange("(c p) o -> p (c o)",
                                                         p=128))
            b1t = consts.tile([128, uc_n], F32)
            nc.sync.dma_start(out=b1t, in_=b1.rearrange("(c p) -> p c", p=128))
            b2t = consts.tile([128, uc_n], F32)
            nc.sync.dma_start(out=b2t, in_=b2.rearrange("(c p) -> p c", p=128))
            b12t = consts.tile([128, uc_n], F32)
            nc.vector.tensor_add(b12t, b1t, b2t)

            qnat = consts.tile([nb, d], F32)
            nc.sync.dma_start(out=qnat, in_=query)
            qt = consts.tile([128, dc_n, nb], F32)
            bias_ub = consts.tile([128, uc_n, nb], F32)

            # --- q_proj = (query @ W1).T + b1 + b2, as per-partition bias ---
            with tc_.tile_pool(name="pre_ps", bufs=2, space="PSUM") as pre_ps, \
                 tc_.tile_pool(name="qp_ps", bufs=2, space="PSUM") as qp_ps, \
                 tc_.tile_pool(name="w1_pool", bufs=1) as w1_pool:
                for dc in range(dc_n):
                    pqt = pre_ps.tile([128, nb], F32)
                    nc.tensor.matmul(pqt, qnat[:, dc * 128:(dc + 1) * 128],
                                     idf4, start=True, stop=True)
                    nc.vector.tensor_copy(qt[:, dc, :], pqt)
                w1sb = w1_pool.tile([128, dc_n, u], F32)
                nc.sync.dma_start(out=w1sb,
                                  in_=w1.rearrange("(c p) u -> p c u", p=128))
                for uc in range(uc_n):
                    qp = qp_ps.tile([128, nb], F32)
                    for dc in range(dc_n):
                        nc.tensor.matmul(qp,
                                         w1sb[:, dc, uc * 128:(uc + 1) * 128],
                                         qt[:, dc, :],
                                         start=(dc == 0), stop=(dc == dc_n - 1))
                    nc.vector.tensor_scalar_add(bias_ub[:, uc, :], qp,
                                                b12t[:, uc:uc + 1])
                if debug_taps:
                    nc.sync.dma_start(out=dbg_bias, in_=bias_ub)

            # --- main loop ---
            with tc_.tile_pool(name="vn", bufs=2) as vn_pool, \
                 tc_.tile_pool(name="vt", bufs=2) as vt_pool, \
                 tc_.tile_pool(name="sc", bufs=3) as sc_pool, \
                 tc_.tile_pool(name="vn2", bufs=2) as vn2_pool, \
                 tc_.tile_pool(name="smalls", bufs=2) as smalls, \
                 tc_.tile_pool(name="ps_tr", bufs=2, space="PSUM") as ps_tr, \
                 tc_.tile_pool(name="ps_v", bufs=2, space="PSUM") as ps_v, \
                 tc_.tile_pool(name="ps_l", bufs=1, space="PSUM") as ps_l, \
                 tc_.tile_pool(name="ps_wt", bufs=1, space="PSUM") as ps_wt, \
                 tc_.tile_pool(name="ps_ctx", bufs=1, space="PSUM") as ps_ctx:
                for b in range(nb):
                    logits = smalls.tile([1, t_len], F32)
                    for tci in range(ntc):
                        t0 = tci * tc
                        vn = vn_pool.tile([128, s_sub, d], mmdt)
                        src = values[b, t0:t0 + tc, :].rearrange(
                            "(s p) d -> p s d", p=128)
                        if use_bf16:
                            nc.gpsimd.dma_start(out=vn, in_=src)
                        else:
                            nc.sync.dma_start(out=vn, in_=src)
                        vt = vt_pool.tile([128, dc_n, tc], mmdt)
                        for s in range(s_sub):
                            for dc in range(dc_n):
                                pt = ps_tr.tile([128, 128], mmdt)
                                nc.tensor.transpose(
                                    pt, vn[:, s, dc * 128:(dc + 1) * 128],
                                    ident)
                                nc.vector.tensor_copy(
                                    vt[:, dc, s * 128:(s + 1) * 128], pt)
                        if debug_taps and b == 0 and tci == 0:
                            vt_f = smalls.tile([128, dc_n, tc], F32)
                            nc.vector.tensor_copy(vt_f, vt)
                            nc.sync.dma_start(out=dbg_vt, in_=vt_f)
                        pl = ps_l.tile([1, tc], F32)
                        for uc in range(uc_n):
                            pv = ps_v.tile([128, tc], F32)
                            for dc in range(dc_n):
                                nc.tensor.matmul(
                                    pv,
                                    w2sb[:, dc, uc * 128:(uc + 1) * 128],
                                    vt[:, dc, :],
                                    start=(dc == 0), stop=(dc == dc_n - 1))
                            sc = sc_pool.tile([128, tc], F32R)
                            nc.scalar.activation(sc, pv, TANH,
                                                 bias=bias_ub[:, uc, b:b + 1])
                            if debug_taps and b == 0 and tci == 0 and uc == 0:
                                sc_f = smalls.tile([128, tc], F32)
                                nc.vector.tensor_copy(sc_f, sc)
                                nc.sync.dma_start(out=dbg_sc, in_=sc_f)
                            nc.tensor.matmul(pl, vsb[:, uc:uc + 1], sc,
                                             start=(uc == 0),
                                             stop=(uc == uc_n - 1))
                        nc.vector.tensor_copy(logits[:, t0:t0 + tc], pl)

                    # softmax over T on one partition
                    negm = smalls.tile([1, 1], F32)
                    nc.vector.reduce_max(negm, logits, axis=AXX, negate=True)
                    wexp = smalls.tile([1, t_len], F32)
                    sume = smalls.tile([1, 1], F32)
                    nc.scalar.activation(wexp, logits, EXP, bias=negm,
                                         accum_out=sume)
                    rden = smalls.tile([1, 1], F32)
                    nc.vector.reciprocal(rden, sume)
                    if debug_taps:
                        nc.sync.dma_start(out=dbg_logits[b:b + 1, :],
                                          in_=logits)
                        nc.sync.dma_start(out=dbg_sume[b:b + 1, 0:1],
                                          in_=sume)
                        nc.sync.dma_start(out=dbg_sume[b:b + 1, 1:2],
                                          in_=negm)
                    wnorm = smalls.tile([1, t_len], F32)
                    nc.vector.tensor_scalar_mul(wnorm, wexp, rden)
                    nc.sync.dma_start(out=aw_out[b:b + 1, :], in_=wnorm)

                    # transpose weights to [t partitions, 1] via K=1 matmul
                    wt = smalls.tile([128, nt128], F32R)
                    for c in range(nt128):
                        pwt = ps_wt.tile([128, 1], F32)
                        nc.tensor.matmul(pwt,
                                         wnorm[0:1, c * 128:(c + 1) * 128],
                                         ones11, start=True, stop=True)
                        nc.vector.tensor_copy(wt[:, c:c + 1], pwt)

                    # context = w.T @ values (second streaming pass)
                    pctx = ps_ctx.tile([1, d], F32)
                    for tci in range(ntc):
                        t0 = tci * tc
                        vn2 = vn2_pool.tile([128, s_sub, d], F32R)
                        src2 = values[b, t0:t0 + tc, :].rearrange(
                            "(s p) d -> p s d", p=128)
                        if use_bf16:
                            nc.gpsimd.dma_start(out=vn2, in_=src2)
                        else:
                            nc.sync.dma_start(out=vn2, in_=src2)
                        for s in range(s_sub):
                            c = tci * s_sub + s
                            for h in range(d // 512):
                                nc.tensor.matmul(
                                    pctx[0:1, h * 512:(h + 1) * 512],
                                    wt[:, c:c + 1],
                                    vn2[:, s, h * 512:(h + 1) * 512],
                                    start=(c == 0), stop=(c == nt128 - 1))
                    ctx_sb = smalls.tile([1, d], F32)
                    nc.vector.tensor_copy(ctx_sb, pctx)
                    nc.sync.dma_start(out=ctx_out[b:b + 1, :], in_=ctx_sb)

    nc.compile()
    return nc


_CACHE = {}


def _get_nc(nb, t_len, d, u, tc, use_bf16):
    key = (nb, t_len, d, u, tc, use_bf16)
    if key not in _CACHE:
        _CACHE[key] = build_nc(nb, t_len, d, u, tc, use_bf16)
    return _CACHE[key]


def kernel(query, values, W1, b1, W2, b2, V, bv=None, **_unused):
    query = np.asarray(query, dtype=np.float32)
    values = np.asarray(values, dtype=np.float32)
    W1 = np.asarray(W1, dtype=np.float32)
    b1 = np.asarray(b1, dtype=np.float32)
    W2 = np.asarray(W2, dtype=np.float32)
    b2 = np.asarray(b2, dtype=np.float32)
    V = np.asarray(V, dtype=np.float32)

    b_tot, t_len, d = values.shape
    u = W1.shape[1]
    assert b_tot % N_CORES == 0
    nb = b_tot // N_CORES

    nc = _get_nc(nb, t_len, d, u, 512 if t_len % 512 == 0 else 256, USE_BF16)

    in_maps = []
    for c in range(N_CORES):
        sl = slice(c * nb, (c + 1) * nb)
        in_maps.append({
            "query": np.ascontiguousarray(query[sl]),
            "values": np.ascontiguousarray(values[sl]),
            "W1": W1, "b1": b1, "W2": W2, "b2": b2, "V": V,
        })

    res = run_bass_kernel_spmd(nc, in_maps, list(range(N_CORES)))

    context = np.concatenate([res.results[c]["ctx_out"]
                              for c in range(N_CORES)], axis=0)
    aw = np.concatenate([res.results[c]["aw_out"]
                         for c in range(N_CORES)], axis=0)
    return context.astype(np.float32), aw[:, :, None].astype(np.float32)
